# revision 27
# baseline (speedup 1.0000x reference)
"""CvT attention block (depthwise conv QKV + MHA) on 8 Trainium2 NeuronCores,
data-parallel over batch (one batch element per core).

Structure (per core, per rep: full DRAM->DRAM recomputation):
  A) depthwise 3x3 convs on DVE (bf16 input, f32 scratch, bf16 out);
     stride-2 k/v convs first so the K/V projections (PE) overlap the q conv.
  B) Q^T/K^T in [co, l] layout, V-hat in [t, (h,65)] layout with a ones
     column per head (softmax denominators fall out of the AV matmul for
     free); PSUM->SBUF evacuations on the Scalar engine (ACT), bf16.
  C) per (l-chunk, head): QK^T into rotating 2-bank PSUM tiles, Exp (ACT)
     into bf16 score tiles, AV accumulation; normalization (reciprocal +
     indicator-matmul broadcast) is software-pipelined one l-chunk behind
     attention so it never blocks the QK PSUM slots; output projection in
     [co, l] layout with the bias fused into the PSUM->SBUF evacuation;
     one bf16 output DMA per rep (host transposes/casts).

Timing reps run in a hardware For_i loop so the measured marginal cost per
rep is true device execution time (program load is amortized). The loop
body holds TWO reps with alternating double-buffered QT/KT/Vh sets, and the
conv+projection of the next rep is emitted interleaved with the attention
l-chunks of the current rep, overlapping DVE conv work with PE/ACT
attention work across rep boundaries.
"""

import contextlib
import numpy as np
import ml_dtypes
from concourse import mybir
import concourse.bacc as bacc
import concourse.tile as tile
from concourse.bass_utils import run_bass_kernel_spmd

F32 = mybir.dt.float32
F32R = mybir.dt.float32r
BF16 = mybir.dt.bfloat16
AFT = mybir.ActivationFunctionType
ALU = mybir.AluOpType

C = 384
T = 3136            # 56*56
TKV = 784           # 28*28
NH = 6
SCALE = C ** (-0.5)
EPS = 1e-5
XP = 56 + T + 56

LC = [(i * 512, min(512, T - i * 512)) for i in range(7)]
T_TILES = [(i * 128, min(128, TKV - i * 128)) for i in range(7)]
TG = [T_TILES[0:2], T_TILES[2:4], T_TILES[4:6], T_TILES[6:7]]   # exp/psum groups

_CACHE = {}


def _conv(nc, eng, xt, ys, ybf, wb, cv, ch, stride):
    """Depthwise 3x3 conv for one 128-channel chunk on `eng` (DVE/GpSimd).
    xt: [128, XP] bf16 padded input (row r col c of the image lives at flat
    56 + r*56 + c, i.e. x3[1+r, c]).
    ys: f32 scratch [128, out_pix]; ybf: bf16 destination.
    wb: [128, 30] tile; tap t of conv cv at col 9*cv+t, bias at col 27+cv.
    """
    w = lambda t: wb[:, 9 * cv + t:9 * cv + t + 1]
    bias = wb[:, 27 + cv:28 + cv]
    x3 = xt[:, 0:3248].rearrange("p (r c) -> p r c", c=56)  # rows -1..56
    if stride == 1:
        ys3 = ys[:].rearrange("p (r c) -> p r c", c=56)
        eng.tensor_scalar(ys[:], xt[:, 56:56 + T], w(4), bias,
                                op0=ALU.mult, op1=ALU.add)
        yield
        for t in (0, 2, 3, 5, 6, 8, 1, 7):
            di, dj = t // 3 - 1, t % 3 - 1
            if dj == 0:
                o = ys[:]
                i = xt[:, 56 + 56 * di:56 + 56 * di + T]
            elif dj < 0:
                o = ys3[:, :, 1:56]
                i = x3[:, 1 + di:57 + di, 0:55]
            else:
                o = ys3[:, :, 0:55]
                i = x3[:, 1 + di:57 + di, 1:56]
            if t == 7:
                o = ybf[:]
                i = xt[:, 56 + 56:56 + 56 + T]
                eng.scalar_tensor_tensor(o, i, w(t), ys[:],
                                               op0=ALU.mult, op1=ALU.add)
            else:
                eng.scalar_tensor_tensor(o, i, w(t), o, op0=ALU.mult,
                                               op1=ALU.add)
            yield
    else:
        ysv = ys[:, 0:TKV]
        ys3 = ysv.rearrange("p (r c) -> p r c", c=28)
        eng.tensor_scalar(ysv, x3[:, 1:57:2, 0:56:2], w(4), bias,
                                op0=ALU.mult, op1=ALU.add)
        yield
        for t in (0, 1, 2, 3, 5, 6, 8, 7):
            di, dj = t // 3 - 1, t % 3 - 1
            if dj == 0:
                o = ysv
                i = x3[:, 1 + di:57 + di:2, 0:56:2]
            elif dj < 0:
                o = ys3[:, :, 1:28]
                i = x3[:, 1 + di:57 + di:2, 1:54:2]
            else:
                o = ysv
                i = x3[:, 1 + di:57 + di:2, 1:56:2]
            if t == 7:
                eng.scalar_tensor_tensor(ybf[:], i, w(t), ysv,
                                               op0=ALU.mult, op1=ALU.add)
            else:
                eng.scalar_tensor_tensor(o, i, w(t), o, op0=ALU.mult,
                                               op1=ALU.add)
            yield


def _emit(nc, tc, ctx, d, reps, use_loop=True):
    pers = ctx.enter_context(tc.tile_pool(name="pers", bufs=1))

    wq = [pers.tile([128, C], BF16, tag=f"wq{i}", name=f"wq{i}") for i in range(3)]
    wk = [pers.tile([128, C], BF16, tag=f"wk{i}", name=f"wk{i}") for i in range(3)]
    wvp = [pers.tile([128, NH * 65], BF16, tag=f"wvp{i}", name=f"wvp{i}")
           for i in range(3)]
    wpj = [pers.tile([128, C], BF16, tag=f"wpj{i}", name=f"wpj{i}")
           for i in range(3)]
    ind6 = [pers.tile([6, 128], F32R, tag=f"ind6{i}", name=f"ind6{i}")
            for i in range(3)]
    wb = [pers.tile([128, 30], F32, tag=f"wb{i}", name=f"wb{i}")
          for i in range(3)]
    bpj = pers.tile([128, 3], F32, tag="bpj", name="bpj")
    QT = [[pers.tile([128, T], BF16, tag=f"QT{b}{i}", name=f"QT{b}{i}")
           for i in range(3)] for b in range(2)]
    KT = [[pers.tile([128, TKV], BF16, tag=f"KT{b}{i}", name=f"KT{b}{i}")
           for i in range(3)] for b in range(2)]
    Vh = [[pers.tile([128, NH * 65], BF16, tag=f"Vh{b}{i}", name=f"Vh{b}{i}")
           for i in range(7)] for b in range(2)]
    outT = pers.tile([128, 3 * T], BF16, tag="outT", name="outT")
    rcf2 = pers.tile([33, NH * 512], F32, tag="rcf2", name="rcf2")
    rc6 = pers.tile([6, 512], F32, tag="rc6", name="rc6")
    rc6r = pers.tile([6, 512], F32R, tag="rc6r", name="rc6r")

    for i in range(3):
        nc.sync.dma_start(wq[i][:], d["wq"][i * 128:(i + 1) * 128, :])
        nc.sync.dma_start(wk[i][:], d["wk"][i * 128:(i + 1) * 128, :])
        nc.sync.dma_start(wvp[i][:], d["wvp"][i * 128:(i + 1) * 128, :])
        nc.sync.dma_start(wpj[i][:], d["wpj"][i * 128:(i + 1) * 128, :])
        nc.sync.dma_start(ind6[i][:], d["ind6"][i])
        nc.sync.dma_start(wb[i][:], d["wb"][i])
    nc.sync.dma_start(bpj[:], d["bpj"])

    # shared pools (one set; conv_piece / attention calls rotate slots)
    ypool = ctx.enter_context(tc.tile_pool(name="ypool", bufs=1))
    yq = [ypool.tile([128, T], BF16, tag=f"yq{i}", name=f"yq{i}")
          for i in range(3)]
    yk = [ypool.tile([128, TKV], BF16, tag=f"yk{i}", name=f"yk{i}")
          for i in range(3)]
    yv = [ypool.tile([128, TKV], BF16, tag=f"yv{i}", name=f"yv{i}")
          for i in range(3)]
    xpool = ctx.enter_context(tc.tile_pool(name="xpool", bufs=3))
    spool = ctx.enter_context(tc.tile_pool(name="spool", bufs=1))
    psB = ctx.enter_context(tc.tile_pool(name="psB", bufs=2, space="PSUM"))
    psQK = ctx.enter_context(tc.tile_pool(name="psQK", bufs=2, space="PSUM"))
    psO = ctx.enter_context(tc.tile_pool(name="psO", bufs=1, space="PSUM"))
    psP = ctx.enter_context(tc.tile_pool(name="psP", bufs=1, space="PSUM"))
    etp = ctx.enter_context(tc.tile_pool(name="etp", bufs=8))
    otp = ctx.enter_context(tc.tile_pool(name="otp", bufs=4))

    xts = {}

    def conv_gen(b):
        """Generator emitting conv+projection for buffer set b, yielding
        after each DVE op so attention can pace the interleaving."""
        for ch in range(3):
            xt = xpool.tile([128, XP], BF16, tag="x", name="x")
            nc.sync.dma_start(xt[:], d["xp"][ch * 128:(ch + 1) * 128, :])
            ys = spool.tile([128, TKV], F32, tag="ysg", name="ysg")
            yield from _conv(nc, nc.vector, xt, ys, yk[ch], wb[ch], 1, ch, 2)
            ys = spool.tile([128, TKV], F32, tag="ysg", name="ysg")
            yield from _conv(nc, nc.vector, xt, ys, yv[ch], wb[ch], 2, ch, 2)
            xts[(b, ch)] = xt
        # K^T projection: [co, t]
        for co in range(3):
            for (to, ts) in ((0, 512), (512, 272)):
                p = psB.tile([128, 512], F32, tag="psB", name="psB")
                for ch in range(3):
                    nc.tensor.matmul(
                        p[:, 0:ts],
                        wk[ch][:, co * 128:(co + 1) * 128],
                        yk[ch][:, to:to + ts],
                        start=(ch == 0), stop=(ch == 2))
                nc.scalar.copy(KT[b][co][:, to:to + ts], p[:, 0:ts])
        # V-hat: [t, (h,65)] with ones column
        for ti, (to, ts) in enumerate(T_TILES):
            p = psB.tile([128, 512], F32, tag="psB", name="psB")
            for ch in range(3):
                nc.tensor.matmul(
                    p[0:ts, 0:NH * 65],
                    yv[ch][:, to:to + ts], wvp[ch][:],
                    start=(ch == 0), stop=(ch == 2))
            nc.scalar.copy(Vh[b][ti][0:ts, :], p[0:ts, 0:NH * 65])
            nc.vector.memset(Vh[b][ti][0:ts, 64:NH * 65:65], 1.0)
        yield
        # q convs
        for ch in range(3):
            ys = spool.tile([128, T], F32, tag="ys", name="ys")
            yield from _conv(nc, nc.vector, xts[(b, ch)], ys, yq[ch],
                             wb[ch], 0, ch, 1)
        # Q^T projection: [co, l], 512-wide windows
        for co in range(3):
            for (lo, ls) in LC:
                p = psB.tile([128, 512], F32, tag="psB", name="psB")
                for ch in range(3):
                    nc.tensor.matmul(
                        p[0:128, 0:ls],
                        wq[ch][:, co * 128:(co + 1) * 128],
                        yq[ch][:, lo:lo + ls],
                        start=(ch == 0), stop=(ch == 2))
                nc.scalar.copy(QT[b][co][:, lo:lo + ls], p[:, 0:ls])

    def conv_proj(b):
        for _ in conv_gen(b):
            pass

    def norm_proj(lo, ls, OTb, rr):
        # gather denominators -> 6 partitions, recip, bcast, divide
        rin = rcf2[rr * 32:rr * 32 + 1, :].rearrange("p (g l) -> p g l", l=512)
        nc.sync.dma_start(rc6[0:6, 0:ls], rin[:, :, 0:ls])
        with nc.allow_low_precision(reason="f32r recip"):
            nc.vector.reciprocal(rc6r[0:6, 0:ls], rc6[0:6, 0:ls])
        for ch in range(3):
            rbp = psP.tile([128, 512], F32, tag="psP", name="psP")
            nc.tensor.matmul(rbp[:, 0:ls],
                             ind6[ch][:], rc6r[0:6, 0:ls],
                             start=True, stop=True)
            nc.vector.tensor_mul(OTb[:, ch * 512:ch * 512 + ls],
                                 OTb[:, ch * 512:ch * 512 + ls],
                                 rbp[:, 0:ls])
        # output projection: out^T[co, l] += bias, bf16
        for co in range(3):
            p = psP.tile([128, 512], F32, tag="psP", name="psP")
            for ch in range(3):
                nc.tensor.matmul(
                    p[:, 0:ls], wpj[ch][:, co * 128:(co + 1) * 128],
                    OTb[:, ch * 512:ch * 512 + ls],
                    start=(ch == 0), stop=(ch == 2))
            nc.vector.tensor_scalar(
                outT[:, co * T + lo:co * T + lo + ls], p[:, 0:ls],
                1.0, bpj[:, co:co + 1], op0=ALU.mult, op1=ALU.add)

    def attention(b, gen=None):
        """Attention over buffer set b; `gen` (next rep's conv_gen) is
        consumed ~2 DVE ops per head so conv work interleaves finely with
        the attention evacuations in DVE's in-order stream."""
        pending = None
        for li, (lo, ls) in enumerate(LC):
            OTb = otp.tile([128, 1536], BF16, tag="otb", name="otb")
            rr = li % 2
            for h in range(NH):
                c2, po = h // 2, 64 * (h % 2)
                ets = []
                for g, tg in enumerate(TG):
                    p = psQK.tile([128, 1024], F32, tag="psQK", name="psQK")
                    for k, (to, ts) in enumerate(tg):
                        nc.tensor.matmul(p[0:ts, k * 512:k * 512 + ls],
                                         KT[b][c2][po:po + 64, to:to + ts],
                                         QT[b][c2][po:po + 64, lo:lo + ls],
                                         start=True, stop=True)
                    et = etp.tile([128, 1024], BF16, tag="et", name="et")
                    wid = (len(tg) - 1) * 512 + ls
                    nc.scalar.activation(et[:, 0:wid], p[:, 0:wid],
                                         AFT.Exp, scale=float(SCALE))
                    ets.append(et)
                po2 = psO.tile([65, 512], F32, tag="psO", name="psO")
                for ti, (to, ts) in enumerate(T_TILES):
                    g, k = ti // 2, ti % 2
                    nc.tensor.matmul(
                        po2[:, :ls], Vh[b][ti][0:ts, h * 65:(h + 1) * 65],
                        ets[g][0:ts, k * 512:k * 512 + ls],
                        start=(ti == 0), stop=(ti == 6))
                nc.vector.tensor_copy(
                    rcf2[rr * 32:rr * 32 + 1, h * 512:h * 512 + ls],
                    po2[64:65, :ls])
                nc.vector.tensor_copy(
                    OTb[po:po + 64, c2 * 512:c2 * 512 + ls], po2[0:64, :ls])
                if gen is not None:
                    next(gen, None)
                    next(gen, None)
                if h == 2 and pending is not None:
                    norm_proj(*pending)
                    pending = None
            pending = (lo, ls, OTb, rr)
        if gen is not None:
            for _ in gen:
                pass
        norm_proj(*pending)
        dst = d["out"].rearrange("(a p) l -> p a l", p=128)
        src = outT[:].rearrange("p (a l) -> p a l", l=T)
        nc.sync.dma_start(dst, src)

    if use_loop and reps > 1:
        assert reps % 2 == 1, "loop path needs odd reps"
        K = (reps - 1) // 2
        conv_proj(0)
        if K > 0:
            with tc.For_i(0, K, 1,
                          hint_engines=(mybir.EngineType.PE,
                                        mybir.EngineType.DVE,
                                        mybir.EngineType.Activation)) as _i:
                attention(0, gen=conv_gen(1))
                attention(1, gen=conv_gen(0))
        attention(0)
    else:
        for rep in range(reps):
            conv_proj(0)
            attention(0)


def _build(reps=1):
    if reps in _CACHE:
        return _CACHE[reps]
    nc = bacc.Bacc("TRN2", target_bir_lowering=False, debug=False)
    d = {
        "xp": nc.dram_tensor("xp", [C, XP], BF16, kind="ExternalInput").ap(),
        "wb": nc.dram_tensor("wb", [3, 128, 30], F32, kind="ExternalInput").ap(),
        "wq": nc.dram_tensor("wq", [C, C], BF16, kind="ExternalInput").ap(),
        "wk": nc.dram_tensor("wk", [C, C], BF16, kind="ExternalInput").ap(),
        "wvp": nc.dram_tensor("wvp", [C, NH * 65], BF16,
                              kind="ExternalInput").ap(),
        "wpj": nc.dram_tensor("wpj", [C, C], BF16, kind="ExternalInput").ap(),
        "ind6": nc.dram_tensor("ind6", [3, 6, 128], F32R,
                               kind="ExternalInput").ap(),
        "bpj": nc.dram_tensor("bpj", [128, 3], F32, kind="ExternalInput").ap(),
        "out": nc.dram_tensor("out", [C, T], BF16, kind="ExternalOutput").ap(),
    }
    with tile.TileContext(nc) as tc:
        with contextlib.ExitStack() as ctx:
            _emit(nc, tc, ctx, d, reps)
    nc.compile()
    _CACHE[reps] = nc
    return nc


def _host_prep(x, conv_q, conv_k, conv_v, bn_q, bn_k, bn_v, Wq, Wk, Wv,
               Wproj, bproj):
    B = x.shape[0]
    x = np.asarray(x, np.float32)
    xp = np.zeros((B, C, XP), ml_dtypes.bfloat16)
    xp[:, :, 56:56 + T] = np.ascontiguousarray(
        x.transpose(0, 2, 1)).astype(ml_dtypes.bfloat16)

    wb = np.zeros((3, 128, 30), np.float32)
    for cv, (w, bn) in enumerate(((conv_q, bn_q), (conv_k, bn_k),
                                  (conv_v, bn_v))):
        g, b, m, v = [np.asarray(bn[i], np.float64) for i in range(4)]
        a = g / np.sqrt(v + EPS)
        bias = (b - m * a).astype(np.float32)
        wh = (np.asarray(w, np.float64).reshape(C, 9) * a[:, None]).astype(
            np.float32)
        for ch in range(3):
            wb[ch, :, 9 * cv:9 * cv + 9] = wh[ch * 128:(ch + 1) * 128]
            wb[ch, :, 27 + cv] = bias[ch * 128:(ch + 1) * 128]

    wvp = np.zeros((C, NH * 65), np.float32)
    Wv = np.asarray(Wv, np.float32)
    for h in range(NH):
        wvp[:, h * 65:h * 65 + 64] = Wv[:, h * 64:(h + 1) * 64]

    ind6 = np.zeros((3, 6, 128), np.float32)
    for ch in range(3):
        ind6[ch, 2 * ch, 0:64] = 1.0
        ind6[ch, 2 * ch + 1, 64:128] = 1.0

    bpj = np.zeros((128, 3), np.float32)
    bp = np.asarray(bproj, np.float32)
    for co in range(3):
        bpj[:, co] = bp[co * 128:(co + 1) * 128]

    bf = ml_dtypes.bfloat16
    return {
        "xp": xp,
        "wb": wb,
        "wq": np.asarray(Wq, np.float32).astype(bf),
        "wk": np.asarray(Wk, np.float32).astype(bf),
        "wvp": wvp.astype(bf),
        "wpj": np.asarray(Wproj, np.float32).astype(bf),
        "ind6": ind6,
        "bpj": bpj,
    }


def kernel(x, h, w, conv_q, conv_k, conv_v, bn_q, bn_k, bn_v, Wq, Wk, Wv,
           Wproj, bproj, _reps=1, _nc=None):
    B = x.shape[0]
    nc = _nc if _nc is not None else _build(_reps)
    hp = _host_prep(x, conv_q, conv_k, conv_v, bn_q, bn_k, bn_v, Wq, Wk, Wv,
                    Wproj, bproj)
    shared = {k: v for k, v in hp.items() if k != "xp"}
    in_maps = [dict(shared, xp=hp["xp"][b]) for b in range(B)]
    res = run_bass_kernel_spmd(nc, in_maps, core_ids=list(range(B)))
    out = np.stack([np.asarray(res.results[b]["out"]).astype(np.float32).T
                    for b in range(B)], axis=0)
    return out


# revision 28
# speedup vs baseline: 1.2417x; 1.2417x over previous
"""CvT attention block (depthwise conv QKV + MHA) on 8 Trainium2 NeuronCores,
data-parallel over batch (one batch element per core).

Structure (per core, per rep: full DRAM->DRAM recomputation):
  A) depthwise 3x3 convs on DVE (bf16 input, f32 scratch, bf16 out);
     stride-2 k/v convs first so the K/V projections (PE) overlap the q conv.
  B) Q^T/K^T in [co, l] layout, V-hat in [t, (h,65)] layout with a ones
     column per head (softmax denominators fall out of the AV matmul for
     free); PSUM->SBUF evacuations on the Scalar engine (ACT), bf16.
  C) per (l-chunk, head): QK^T into rotating 2-bank PSUM tiles, Exp (ACT)
     into bf16 score tiles, AV accumulation; normalization (reciprocal +
     indicator-matmul broadcast) is software-pipelined one l-chunk behind
     attention so it never blocks the QK PSUM slots; output projection in
     [co, l] layout with the bias fused into the PSUM->SBUF evacuation;
     one bf16 output DMA per rep (host transposes/casts).

Timing reps run in a hardware For_i loop so the measured marginal cost per
rep is true device execution time (program load is amortized). The loop
body holds TWO reps with alternating double-buffered QT/KT/Vh sets, and the
conv+projection of the next rep is emitted interleaved with the attention
l-chunks of the current rep, overlapping DVE conv work with PE/ACT
attention work across rep boundaries.
"""

import contextlib
import numpy as np
import ml_dtypes
from concourse import mybir
import concourse.bacc as bacc
import concourse.tile as tile
from concourse.bass_utils import run_bass_kernel_spmd

F32 = mybir.dt.float32
F32R = mybir.dt.float32r
BF16 = mybir.dt.bfloat16
AFT = mybir.ActivationFunctionType
ALU = mybir.AluOpType

C = 384
T = 3136            # 56*56
TKV = 784           # 28*28
NH = 6
SCALE = C ** (-0.5)
EPS = 1e-5
XP = 56 + T + 56

LC = [(i * 512, min(512, T - i * 512)) for i in range(7)]
T_TILES = [(i * 128, min(128, TKV - i * 128)) for i in range(7)]
TG = [T_TILES[0:2], T_TILES[2:4], T_TILES[4:6], T_TILES[6:7]]   # exp/psum groups

_CACHE = {}


def _conv(nc, eng, xt, ys, ybf, wb, cv, ch, stride):
    """Depthwise 3x3 conv for one 128-channel chunk on `eng` (DVE/GpSimd).
    xt: [128, XP] bf16 padded input (row r col c of the image lives at flat
    56 + r*56 + c, i.e. x3[1+r, c]).
    ys: f32 scratch [128, out_pix]; ybf: bf16 destination.
    wb: [128, 30] tile; tap t of conv cv at col 9*cv+t, bias at col 27+cv.
    """
    w = lambda t: wb[:, 9 * cv + t:9 * cv + t + 1]
    bias = wb[:, 27 + cv:28 + cv]
    x3 = xt[:, 0:3248].rearrange("p (r c) -> p r c", c=56)  # rows -1..56
    if stride == 1:
        ys3 = ys[:].rearrange("p (r c) -> p r c", c=56)
        eng.tensor_scalar(ys[:], xt[:, 56:56 + T], w(4), bias,
                                op0=ALU.mult, op1=ALU.add)
        yield
        for t in (0, 2, 3, 5, 6, 8, 1, 7):
            di, dj = t // 3 - 1, t % 3 - 1
            if dj == 0:
                o = ys[:]
                i = xt[:, 56 + 56 * di:56 + 56 * di + T]
            elif dj < 0:
                o = ys3[:, :, 1:56]
                i = x3[:, 1 + di:57 + di, 0:55]
            else:
                o = ys3[:, :, 0:55]
                i = x3[:, 1 + di:57 + di, 1:56]
            if t == 7:
                o = ybf[:]
                i = xt[:, 56 + 56:56 + 56 + T]
                eng.scalar_tensor_tensor(o, i, w(t), ys[:],
                                               op0=ALU.mult, op1=ALU.add)
            else:
                eng.scalar_tensor_tensor(o, i, w(t), o, op0=ALU.mult,
                                               op1=ALU.add)
            yield
    else:
        ysv = ys[:, 0:TKV]
        ys3 = ysv.rearrange("p (r c) -> p r c", c=28)
        eng.tensor_scalar(ysv, x3[:, 1:57:2, 0:56:2], w(4), bias,
                                op0=ALU.mult, op1=ALU.add)
        yield
        for t in (0, 1, 2, 3, 5, 6, 8, 7):
            di, dj = t // 3 - 1, t % 3 - 1
            if dj == 0:
                o = ysv
                i = x3[:, 1 + di:57 + di:2, 0:56:2]
            elif dj < 0:
                o = ys3[:, :, 1:28]
                i = x3[:, 1 + di:57 + di:2, 1:54:2]
            else:
                o = ysv
                i = x3[:, 1 + di:57 + di:2, 1:56:2]
            if t == 7:
                eng.scalar_tensor_tensor(ybf[:], i, w(t), ysv,
                                               op0=ALU.mult, op1=ALU.add)
            else:
                eng.scalar_tensor_tensor(o, i, w(t), o, op0=ALU.mult,
                                               op1=ALU.add)
            yield


def _emit(nc, tc, ctx, d, reps, use_loop=True):
    pers = ctx.enter_context(tc.tile_pool(name="pers", bufs=1))

    wq = [pers.tile([128, C], BF16, tag=f"wq{i}", name=f"wq{i}") for i in range(3)]
    wk = [pers.tile([128, C], BF16, tag=f"wk{i}", name=f"wk{i}") for i in range(3)]
    wvp = [pers.tile([128, NH * 65], BF16, tag=f"wvp{i}", name=f"wvp{i}")
           for i in range(3)]
    wpj = [pers.tile([128, C], BF16, tag=f"wpj{i}", name=f"wpj{i}")
           for i in range(3)]
    ind6 = [pers.tile([6, 128], F32R, tag=f"ind6{i}", name=f"ind6{i}")
            for i in range(3)]
    wb = [pers.tile([128, 30], F32, tag=f"wb{i}", name=f"wb{i}")
          for i in range(3)]
    bpj = pers.tile([128, 3], F32, tag="bpj", name="bpj")
    QT = [[pers.tile([128, T], BF16, tag=f"QT{b}{i}", name=f"QT{b}{i}")
           for i in range(3)] for b in range(2)]
    KT = [[pers.tile([128, TKV], BF16, tag=f"KT{b}{i}", name=f"KT{b}{i}")
           for i in range(3)] for b in range(2)]
    Vh = [[pers.tile([128, NH * 65], BF16, tag=f"Vh{b}{i}", name=f"Vh{b}{i}")
           for i in range(7)] for b in range(2)]
    outT = pers.tile([128, 3 * T], BF16, tag="outT", name="outT")
    rcf2 = pers.tile([33, NH * 512], F32, tag="rcf2", name="rcf2")
    rc6 = pers.tile([6, 512], F32, tag="rc6", name="rc6")
    rc6r = pers.tile([6, 512], F32R, tag="rc6r", name="rc6r")

    for i in range(3):
        nc.sync.dma_start(wq[i][:], d["wq"][i * 128:(i + 1) * 128, :])
        nc.sync.dma_start(wk[i][:], d["wk"][i * 128:(i + 1) * 128, :])
        nc.sync.dma_start(wvp[i][:], d["wvp"][i * 128:(i + 1) * 128, :])
        nc.sync.dma_start(wpj[i][:], d["wpj"][i * 128:(i + 1) * 128, :])
        nc.sync.dma_start(ind6[i][:], d["ind6"][i])
        nc.sync.dma_start(wb[i][:], d["wb"][i])
    nc.sync.dma_start(bpj[:], d["bpj"])

    # shared pools (one set; conv_piece / attention calls rotate slots)
    ypool = ctx.enter_context(tc.tile_pool(name="ypool", bufs=1))
    yq = [ypool.tile([128, T], BF16, tag=f"yq{i}", name=f"yq{i}")
          for i in range(3)]
    yk = [ypool.tile([128, TKV], BF16, tag=f"yk{i}", name=f"yk{i}")
          for i in range(3)]
    yv = [ypool.tile([128, TKV], BF16, tag=f"yv{i}", name=f"yv{i}")
          for i in range(3)]
    xpool = ctx.enter_context(tc.tile_pool(name="xpool", bufs=3))
    spool = ctx.enter_context(tc.tile_pool(name="spool", bufs=1))
    psB = ctx.enter_context(tc.tile_pool(name="psB", bufs=2, space="PSUM"))
    psQK = ctx.enter_context(tc.tile_pool(name="psQK", bufs=2, space="PSUM"))
    psO = ctx.enter_context(tc.tile_pool(name="psO", bufs=1, space="PSUM"))
    psP = ctx.enter_context(tc.tile_pool(name="psP", bufs=1, space="PSUM"))
    etp = ctx.enter_context(tc.tile_pool(name="etp", bufs=6))
    otp = ctx.enter_context(tc.tile_pool(name="otp", bufs=3))

    xts = {}

    def conv_gen(b):
        """Generator emitting conv+projection for buffer set b, yielding
        after each DVE op so attention can pace the interleaving."""
        for ch in range(3):
            xt = xpool.tile([128, XP], BF16, tag="x", name="x")
            nc.sync.dma_start(xt[:], d["xp"][ch * 128:(ch + 1) * 128, :])
            ys = spool.tile([128, TKV], F32, tag="ysg", name="ysg")
            yield from _conv(nc, nc.vector, xt, ys, yk[ch], wb[ch], 1, ch, 2)
            ys = spool.tile([128, TKV], F32, tag="ysg", name="ysg")
            yield from _conv(nc, nc.vector, xt, ys, yv[ch], wb[ch], 2, ch, 2)
            xts[(b, ch)] = xt
        # K^T projection: [co, t]
        for co in range(3):
            for (to, ts) in ((0, 512), (512, 272)):
                p = psB.tile([128, 512], F32, tag="psB", name="psB")
                for ch in range(3):
                    nc.tensor.matmul(
                        p[:, 0:ts],
                        wk[ch][:, co * 128:(co + 1) * 128],
                        yk[ch][:, to:to + ts],
                        start=(ch == 0), stop=(ch == 2))
                nc.scalar.copy(KT[b][co][:, to:to + ts], p[:, 0:ts])
        # V-hat: [t, (h,65)] with ones column
        for ti, (to, ts) in enumerate(T_TILES):
            p = psB.tile([128, 512], F32, tag="psB", name="psB")
            for ch in range(3):
                nc.tensor.matmul(
                    p[0:ts, 0:NH * 65],
                    yv[ch][:, to:to + ts], wvp[ch][:],
                    start=(ch == 0), stop=(ch == 2))
            nc.scalar.copy(Vh[b][ti][0:ts, :], p[0:ts, 0:NH * 65])
            nc.vector.memset(Vh[b][ti][0:ts, 64:NH * 65:65], 1.0)
        yield
        # q convs
        for ch in range(3):
            ys = spool.tile([128, T], F32, tag="ys", name="ys")
            yield from _conv(nc, nc.vector, xts[(b, ch)], ys, yq[ch],
                             wb[ch], 0, ch, 1)
        # Q^T projection: [co, l], 512-wide windows
        for co in range(3):
            for (lo, ls) in LC:
                p = psB.tile([128, 512], F32, tag="psB", name="psB")
                for ch in range(3):
                    nc.tensor.matmul(
                        p[0:128, 0:ls],
                        wq[ch][:, co * 128:(co + 1) * 128],
                        yq[ch][:, lo:lo + ls],
                        start=(ch == 0), stop=(ch == 2))
                nc.scalar.copy(QT[b][co][:, lo:lo + ls], p[:, 0:ls])

    def conv_proj(b):
        for _ in conv_gen(b):
            pass

    def norm_proj(lo, ls, OTb, rr):
        # gather denominators -> 6 partitions, recip, bcast, divide
        rin = rcf2[rr * 32:rr * 32 + 1, :].rearrange("p (g l) -> p g l", l=512)
        nc.sync.dma_start(rc6[0:6, 0:ls], rin[:, :, 0:ls])
        with nc.allow_low_precision(reason="f32r recip"):
            nc.vector.reciprocal(rc6r[0:6, 0:ls], rc6[0:6, 0:ls])
        for ch in range(3):
            rbp = psP.tile([128, 512], F32, tag="psP", name="psP")
            nc.tensor.matmul(rbp[:, 0:ls],
                             ind6[ch][:], rc6r[0:6, 0:ls],
                             start=True, stop=True)
            nc.vector.tensor_mul(OTb[:, ch * 512:ch * 512 + ls],
                                 OTb[:, ch * 512:ch * 512 + ls],
                                 rbp[:, 0:ls])
        # output projection: out^T[co, l] += bias, bf16
        for co in range(3):
            p = psP.tile([128, 512], F32, tag="psP", name="psP")
            for ch in range(3):
                nc.tensor.matmul(
                    p[:, 0:ls], wpj[ch][:, co * 128:(co + 1) * 128],
                    OTb[:, ch * 512:ch * 512 + ls],
                    start=(ch == 0), stop=(ch == 2))
            nc.vector.tensor_scalar(
                outT[:, co * T + lo:co * T + lo + ls], p[:, 0:ls],
                1.0, bpj[:, co:co + 1], op0=ALU.mult, op1=ALU.add)

    def attention(b, gen=None):
        """Attention over buffer set b; `gen` (next rep's conv_gen) is
        consumed ~2 DVE ops per head so conv work interleaves finely with
        the attention evacuations in DVE's in-order stream."""
        pending = None
        for li, (lo, ls) in enumerate(LC):
            OTb = otp.tile([128, 1536], BF16, tag="otb", name="otb")
            rr = li % 2
            for h in range(NH):
                c2, po = h // 2, 64 * (h % 2)
                ets = []
                for g, tg in enumerate(TG):
                    p = psQK.tile([128, 1024], F32, tag="psQK", name="psQK")
                    for k, (to, ts) in enumerate(tg):
                        nc.tensor.matmul(p[0:ts, k * 512:k * 512 + ls],
                                         KT[b][c2][po:po + 64, to:to + ts],
                                         QT[b][c2][po:po + 64, lo:lo + ls],
                                         start=True, stop=True)
                    et = etp.tile([128, 1024], BF16, tag="et", name="et")
                    wid = (len(tg) - 1) * 512 + ls
                    nc.scalar.activation(et[:, 0:wid], p[:, 0:wid],
                                         AFT.Exp, scale=float(SCALE))
                    ets.append(et)
                po2 = psO.tile([65, 512], F32, tag="psO", name="psO")
                for ti, (to, ts) in enumerate(T_TILES):
                    g, k = ti // 2, ti % 2
                    nc.tensor.matmul(
                        po2[:, :ls], Vh[b][ti][0:ts, h * 65:(h + 1) * 65],
                        ets[g][0:ts, k * 512:k * 512 + ls],
                        start=(ti == 0), stop=(ti == 6))
                nc.vector.tensor_copy(
                    rcf2[rr * 32:rr * 32 + 1, h * 512:h * 512 + ls],
                    po2[64:65, :ls])
                nc.vector.tensor_copy(
                    OTb[po:po + 64, c2 * 512:c2 * 512 + ls], po2[0:64, :ls])
                if gen is not None:
                    next(gen, None)
                    next(gen, None)
                if h == 2 and pending is not None:
                    norm_proj(*pending)
                    pending = None
            pending = (lo, ls, OTb, rr)
        if gen is not None:
            for _ in gen:
                pass
        norm_proj(*pending)
        dst = d["out"].rearrange("(a p) l -> p a l", p=128)
        src = outT[:].rearrange("p (a l) -> p a l", l=T)
        nc.sync.dma_start(dst, src)

    if use_loop and reps > 1:
        assert reps % 2 == 1, "loop path needs odd reps"
        K = (reps - 1) // 2
        conv_proj(0)
        if K > 0:
            with tc.For_i(0, K, 1,
                          hint_engines=(mybir.EngineType.PE,
                                        mybir.EngineType.DVE,
                                        mybir.EngineType.Activation)) as _i:
                attention(0, gen=conv_gen(1))
                attention(1, gen=conv_gen(0))
        attention(0)
    else:
        for rep in range(reps):
            conv_proj(0)
            attention(0)


def _build(reps=1):
    if reps in _CACHE:
        return _CACHE[reps]
    nc = bacc.Bacc("TRN2", target_bir_lowering=False, debug=False)
    d = {
        "xp": nc.dram_tensor("xp", [C, XP], BF16, kind="ExternalInput").ap(),
        "wb": nc.dram_tensor("wb", [3, 128, 30], F32, kind="ExternalInput").ap(),
        "wq": nc.dram_tensor("wq", [C, C], BF16, kind="ExternalInput").ap(),
        "wk": nc.dram_tensor("wk", [C, C], BF16, kind="ExternalInput").ap(),
        "wvp": nc.dram_tensor("wvp", [C, NH * 65], BF16,
                              kind="ExternalInput").ap(),
        "wpj": nc.dram_tensor("wpj", [C, C], BF16, kind="ExternalInput").ap(),
        "ind6": nc.dram_tensor("ind6", [3, 6, 128], F32R,
                               kind="ExternalInput").ap(),
        "bpj": nc.dram_tensor("bpj", [128, 3], F32, kind="ExternalInput").ap(),
        "out": nc.dram_tensor("out", [C, T], BF16, kind="ExternalOutput").ap(),
    }
    with tile.TileContext(nc) as tc:
        with contextlib.ExitStack() as ctx:
            _emit(nc, tc, ctx, d, reps)
    nc.compile()
    _CACHE[reps] = nc
    return nc


def _host_prep(x, conv_q, conv_k, conv_v, bn_q, bn_k, bn_v, Wq, Wk, Wv,
               Wproj, bproj):
    B = x.shape[0]
    x = np.asarray(x, np.float32)
    xp = np.zeros((B, C, XP), ml_dtypes.bfloat16)
    xp[:, :, 56:56 + T] = np.ascontiguousarray(
        x.transpose(0, 2, 1)).astype(ml_dtypes.bfloat16)

    wb = np.zeros((3, 128, 30), np.float32)
    for cv, (w, bn) in enumerate(((conv_q, bn_q), (conv_k, bn_k),
                                  (conv_v, bn_v))):
        g, b, m, v = [np.asarray(bn[i], np.float64) for i in range(4)]
        a = g / np.sqrt(v + EPS)
        bias = (b - m * a).astype(np.float32)
        wh = (np.asarray(w, np.float64).reshape(C, 9) * a[:, None]).astype(
            np.float32)
        for ch in range(3):
            wb[ch, :, 9 * cv:9 * cv + 9] = wh[ch * 128:(ch + 1) * 128]
            wb[ch, :, 27 + cv] = bias[ch * 128:(ch + 1) * 128]

    wvp = np.zeros((C, NH * 65), np.float32)
    Wv = np.asarray(Wv, np.float32)
    for h in range(NH):
        wvp[:, h * 65:h * 65 + 64] = Wv[:, h * 64:(h + 1) * 64]

    ind6 = np.zeros((3, 6, 128), np.float32)
    for ch in range(3):
        ind6[ch, 2 * ch, 0:64] = 1.0
        ind6[ch, 2 * ch + 1, 64:128] = 1.0

    bpj = np.zeros((128, 3), np.float32)
    bp = np.asarray(bproj, np.float32)
    for co in range(3):
        bpj[:, co] = bp[co * 128:(co + 1) * 128]

    bf = ml_dtypes.bfloat16
    return {
        "xp": xp,
        "wb": wb,
        "wq": np.asarray(Wq, np.float32).astype(bf),
        "wk": np.asarray(Wk, np.float32).astype(bf),
        "wvp": wvp.astype(bf),
        "wpj": np.asarray(Wproj, np.float32).astype(bf),
        "ind6": ind6,
        "bpj": bpj,
    }


def kernel(x, h, w, conv_q, conv_k, conv_v, bn_q, bn_k, bn_v, Wq, Wk, Wv,
           Wproj, bproj, _reps=1, _nc=None):
    B = x.shape[0]
    nc = _nc if _nc is not None else _build(_reps)
    hp = _host_prep(x, conv_q, conv_k, conv_v, bn_q, bn_k, bn_v, Wq, Wk, Wv,
                    Wproj, bproj)
    shared = {k: v for k, v in hp.items() if k != "xp"}
    in_maps = [dict(shared, xp=hp["xp"][b]) for b in range(B)]
    res = run_bass_kernel_spmd(nc, in_maps, core_ids=list(range(B)))
    out = np.stack([np.asarray(res.results[b]["out"]).astype(np.float32).T
                    for b in range(B)], axis=0)
    return out
